# revision 10
# baseline (speedup 1.0000x reference)
"""Performer (FAVOR+) attention block for Trainium2, 8-core SPMD.

Sharding: core i handles batch n = i//2 and head-group hg = i%2 (8 of 16
heads). All cross-head/cross-batch reductions are local to a core; the only
cross-core step is summing the two per-head-group partial outputs of each
batch (done on host after gather).

Math restructuring (validated vs reference to ~2e-6 rel):
  - P = QR-orthogonalization of projection_matrix (host, numpy - tiny).
  - x_k = key @ Wxk  with Wxk = (D^-1/4 * wk)^T                (per head grp)
  - proj = x @ P  composed into the input projection:  W2 = Wxk @ P_h
  - query path: h = ||x||^2/2 cancels inside exp(delta - max_e delta), so
    only proj_q is needed:  qf = D^-1/2 * exp(proj - max_e proj) + eps
  - key path: kp = exp(proj - (h + diff + ln(D)/2)) + eps, diff = global max
  - out_proj composed through kv:  y = sum_h qfs_h @ (kv_h @ WoutT_h),
    qfs = qf * D_inv  (denominator applied to qf before the kv matmul)
"""
import numpy as np
from contextlib import ExitStack

import concourse.bass as bass
import concourse.tile as tile
from concourse import bacc, mybir
from concourse.bass_utils import run_bass_kernel_spmd
from concourse.masks import make_identity
from concourse import bass_isa

L, N, E, H, D = 4096, 4, 1024, 16, 64
HG = 2                 # head-groups (cores per batch)
HPG = H // HG          # 8 heads per group
DH = HPG * D           # 512 feature columns per group
LT = 128               # rows per l-tile
NLT = L // LT          # 32
SLAB = 512             # l columns per DMA slab
NSLAB = L // SLAB      # 8
JT = E // 128          # 8 contraction tiles

F32 = mybir.dt.float32
F32R = mybir.dt.float32r
EXP = mybir.ActivationFunctionType.Exp
ALU = mybir.AluOpType
AXX = mybir.AxisListType.X
AXC = mybir.AxisListType.C

CM = float(D ** -0.5)
EPS = 1e-6
STAB = 1e-6
B2C = float(0.5 * np.log(D))


def _bc(ap2, g_axis_first, reps):
    """[P, G] -> broadcast AP. g_axis_first=True: [P, G, reps] (repeat inner);
    False: [P, reps, G] (repeat middle)."""
    p_dim, g_dim = ap2.ap[0], ap2.ap[1]
    if g_axis_first:
        return bass.AP(tensor=ap2.tensor, offset=ap2.offset,
                       ap=[p_dim, g_dim, [0, reps]])
    return bass.AP(tensor=ap2.tensor, offset=ap2.offset,
                   ap=[p_dim, [0, reps], g_dim])


def _seg(ap2, g):
    """[P, G*D] tile view -> [P, G, D]."""
    return ap2.rearrange("p (g d) -> p g d", g=g)


def _build_program(use_bias: bool, debug: bool = False):
    nc = bacc.Bacc("TRN2", target_bir_lowering=False, debug=False)

    qT_d = nc.dram_tensor("qT", [E, L], F32R, kind="ExternalInput")
    kT_d = nc.dram_tensor("kT", [E, L], F32R, kind="ExternalInput")
    vT_d = nc.dram_tensor("vT", [E, L], F32R, kind="ExternalInput")
    w2q_d = nc.dram_tensor("w2q", [E, DH], F32R, kind="ExternalInput")
    wxk_d = nc.dram_tensor("wxk", [E, DH], F32R, kind="ExternalInput")
    w2k_d = nc.dram_tensor("w2k", [E, DH], F32R, kind="ExternalInput")
    wv_d = nc.dram_tensor("wv", [E, DH], F32R, kind="ExternalInput")
    woT_d = nc.dram_tensor("woT", [D, HPG * E], F32R, kind="ExternalInput")
    if use_bias:
        bias_d = nc.dram_tensor("biases", [1, 3 * DH], F32R, kind="ExternalInput")
    y_d = nc.dram_tensor("y", [L, E], F32, kind="ExternalOutput")
    if debug:
        dbg_hd = nc.dram_tensor("dbg_hd", [128, NLT * HPG], F32, kind="ExternalOutput")
        dbg_delta0 = nc.dram_tensor("dbg_delta0", [128, DH], F32, kind="ExternalOutput")
        dbg_kp0 = nc.dram_tensor("dbg_kp0", [128, DH], F32, kind="ExternalOutput")
        dbg_v0 = nc.dram_tensor("dbg_v0", [128, DH], F32, kind="ExternalOutput")
        dbg_kv = nc.dram_tensor("dbg_kv", [64, DH], F32, kind="ExternalOutput")
        dbg_ksumb = nc.dram_tensor("dbg_ksumb", [128, DH], F32, kind="ExternalOutput")
        dbg_wkv = nc.dram_tensor("dbg_wkv", [128, 4 * E], F32, kind="ExternalOutput")
        dbg_qfs0 = nc.dram_tensor("dbg_qfs0", [128, DH], F32, kind="ExternalOutput")
        dbg_qt0 = nc.dram_tensor("dbg_qt0", [128, 4 * 128], F32, kind="ExternalOutput")

    qT = qT_d.ap().rearrange("(jt p) l -> p jt l", p=128)
    kT = kT_d.ap().rearrange("(jt p) l -> p jt l", p=128)
    vT = vT_d.ap().rearrange("(jt p) l -> p jt l", p=128)

    with tile.TileContext(nc) as tc, ExitStack() as ctx:
        const = ctx.enter_context(tc.tile_pool(name="const", bufs=1))
        pers = ctx.enter_context(tc.tile_pool(name="pers", bufs=1))

        ident = const.tile([128, 128], F32)
        make_identity(nc, ident)
        ones_col = const.tile([128, 1], F32)
        nc.vector.memset(ones_col, 1.0)
        ones_row = const.tile([1, 128], F32)
        nc.vector.memset(ones_row, 1.0)
        if use_bias:
            bias_sb = const.tile([1, 3, DH], F32R)
            nc.sync.dma_start(bias_sb, bias_d.ap().rearrange("o (t n) -> o t n", t=3))
            ones_row_r = const.tile([1, 128], F32R)
            nc.vector.memset(ones_row_r, 1.0)

        # persistent across phases
        hd_all = pers.tile([128, NLT, HPG], F32)      # h-sums, then 0.5h+b2
        runmax = pers.tile([128, HPG], F32)
        ksumb = pers.tile([128, DH], F32)             # ksum broadcast to rows
        epskss = pers.tile([128, HPG], F32)
        wkv_sb = pers.tile([128, 4, E], F32R)         # stacked head-pair kv@WoutT

        nc.vector.memset(runmax, -1e30)

        # ---------------- phase K1: x_k, proj_k, h, running max --------------
        with tc.tile_pool(name="deltap", bufs=1) as deltap:
            delta_all = deltap.tile([128, NLT, DH], F32)   # stores proj_k

            with tc.tile_pool(name="wk", bufs=1) as wkp, \
                 tc.tile_pool(name="kslab", bufs=2) as kslab, \
                 tc.tile_pool(name="k1tmp", bufs=3) as k1tmp, \
                 tc.tile_pool(name="k1sm", bufs=4) as k1sm, \
                 tc.tile_pool(name="psK1", bufs=2, space="PSUM") as psK1:
                wxk_sb = wkp.tile([128, JT, DH], F32R)
                w2k_sb = wkp.tile([128, JT, DH], F32R)
                nc.sync.dma_start(wxk_sb, wxk_d.ap().rearrange("(jt p) n -> p jt n", p=128))
                nc.sync.dma_start(w2k_sb, w2k_d.ap().rearrange("(jt p) n -> p jt n", p=128))

                for ls in range(NSLAB):
                    slab = kslab.tile([128, JT, SLAB], F32R, tag="slab")
                    nc.sync.dma_start(slab, kT[:, :, ls * SLAB:(ls + 1) * SLAB])
                    for t in range(SLAB // LT):
                        ti = ls * (SLAB // LT) + t
                        px = psK1.tile([128, DH], F32, tag="px")
                        pp = psK1.tile([128, DH], F32, tag="pp")
                        for jt in range(JT):
                            lhsT = slab[:, jt, t * LT:(t + 1) * LT]
                            nc.tensor.matmul(px, lhsT, wxk_sb[:, jt, :],
                                             start=(jt == 0), stop=(not use_bias and jt == JT - 1))
                            nc.tensor.matmul(pp, lhsT, w2k_sb[:, jt, :],
                                             start=(jt == 0), stop=(not use_bias and jt == JT - 1))
                        if use_bias:
                            nc.tensor.matmul(px, ones_row_r, bias_sb[:, 0, :], start=False, stop=True)
                            nc.tensor.matmul(pp, ones_row_r, bias_sb[:, 1, :], start=False, stop=True)
                        # h-sum and per-tile max(proj - 0.5*hsum)
                        sq = k1tmp.tile([128, DH], F32, tag="sq")
                        nc.scalar.activation(sq, px, mybir.ActivationFunctionType.Square)
                        nc.vector.tensor_reduce(hd_all[:, ti, :], _seg(sq, HPG), AXX, ALU.add)
                        nc.scalar.copy(delta_all[:, ti, :], pp)     # ACT: psum->sbuf
                        rm = k1sm.tile([128, HPG], F32, tag="rm")
                        nc.vector.tensor_reduce(rm, _seg(pp, HPG), AXX, ALU.max)
                        td = k1sm.tile([128, HPG], F32, tag="td")
                        nc.vector.scalar_tensor_tensor(out=td, in0=hd_all[:, ti, :],
                                                       scalar=-0.5, in1=rm,
                                                       op0=ALU.mult, op1=ALU.add)
                        nc.vector.tensor_tensor(out=runmax, in0=runmax, in1=td, op=ALU.max)

            # ---------------- phase K1.5: diff, b2, hd2 ----------------------
            with tc.tile_pool(name="k15", bufs=1) as k15:
                diffb = k15.tile([128, HPG], F32)
                nc.gpsimd.partition_all_reduce(diffb, runmax, 128,
                                               bass_isa.ReduceOp.max)
                b2cols = k15.tile([128, HPG], F32)
                nc.vector.tensor_scalar_add(b2cols, diffb, B2C)
                # hd2 = 0.5*hsum + b2   (one op over the whole store)
                nc.vector.scalar_tensor_tensor(
                    out=hd_all.rearrange("p t g -> p (t g)"),
                    in0=hd_all.rearrange("p t g -> p (t g)"),
                    scalar=0.5,
                    in1=_bc(b2cols, False, NLT),
                    op0=ALU.mult, op1=ALU.add)

            # ---------------- phase K2: v, kp, kv/ksum accumulation ----------
            with tc.tile_pool(name="psAcc", bufs=1, space="PSUM") as psAcc:
                kvps = psAcc.tile([64, DH], F32, tag="kv")
                ksps = psAcc.tile([64, HPG], F32, tag="ks")
                with tc.tile_pool(name="wv", bufs=1) as wvp, \
                     tc.tile_pool(name="vslab", bufs=2) as vslab, \
                     tc.tile_pool(name="k2tmp", bufs=3) as k2tmp, \
                     tc.tile_pool(name="psK2", bufs=2, space="PSUM") as psK2:
                    wv_sb = wvp.tile([128, JT, DH], F32R)
                    nc.sync.dma_start(wv_sb, wv_d.ap().rearrange("(jt p) n -> p jt n", p=128))
                    for ls in range(NSLAB):
                        slab = vslab.tile([128, JT, SLAB], F32R, tag="slab")
                        nc.sync.dma_start(slab, vT[:, :, ls * SLAB:(ls + 1) * SLAB])
                        for t in range(SLAB // LT):
                            ti = ls * (SLAB // LT) + t
                            pv = psK2.tile([128, DH], F32, tag="pv")
                            for jt in range(JT):
                                nc.tensor.matmul(pv, slab[:, jt, t * LT:(t + 1) * LT],
                                                 wv_sb[:, jt, :],
                                                 start=(jt == 0), stop=(not use_bias and jt == JT - 1))
                            if use_bias:
                                nc.tensor.matmul(pv, ones_row_r, bias_sb[:, 2, :], start=False, stop=True)
                            v_sb = k2tmp.tile([128, DH], F32, tag="v")
                            nc.scalar.copy(v_sb, pv)                    # ACT
                            kpe = k2tmp.tile([128, DH], F32, tag="kpe")
                            nc.vector.tensor_tensor(
                                out=_seg(kpe, HPG), in0=_seg(delta_all[:, ti, :], HPG),
                                in1=_bc(hd_all[:, ti, :], True, D), op=ALU.subtract)
                            kx = k2tmp.tile([128, DH], F32, tag="kx")
                            nc.scalar.activation(kx, kpe, EXP)          # ACT
                            kp_sb = k2tmp.tile([128, DH], F32, tag="kp")
                            nc.vector.tensor_scalar_add(kp_sb, kx, EPS)
                            if debug and ti == 0:
                                nc.sync.dma_start(dbg_kp0.ap(), kp_sb)
                                nc.sync.dma_start(dbg_v0.ap(), v_sb)
                                nc.sync.dma_start(dbg_delta0.ap(), delta_all[:, 0, :])
                                nc.sync.dma_start(dbg_hd.ap(), hd_all.rearrange("p t g -> p (t g)"))
                            last = (ti == NLT - 1)
                            for h in range(HPG):
                                hs = slice(h * D, (h + 1) * D)
                                # start=True clears the whole PSUM bank, so only
                                # the first matmul touching each accumulator may
                                # set it; later regions overwrite-on-clear.
                                nc.tensor.matmul(kvps[:, hs], kp_sb[:, hs], v_sb[:, hs],
                                                 start=(ti == 0 and h == 0),
                                                 stop=(last and h == HPG - 1))
                                nc.tensor.matmul(ksps[:, h:h + 1], kp_sb[:, hs], ones_col,
                                                 start=(ti == 0 and h == 0),
                                                 stop=(last and h == HPG - 1))

                # ---------------- phase C: compose Wkv, ksum broadcast -------
                with tc.tile_pool(name="cw", bufs=1) as cw, \
                     tc.tile_pool(name="psC", bufs=1, space="PSUM") as psC:
                    woT_sb = cw.tile([64, HPG, E], F32R)
                    nc.sync.dma_start(woT_sb, woT_d.ap().rearrange("d (g e) -> d g e", g=HPG))
                    kv_sb = cw.tile([64, DH], F32)
                    nc.vector.tensor_copy(kv_sb, kvps)
                    ks_sb = cw.tile([64, HPG], F32)
                    nc.vector.tensor_copy(ks_sb, ksps)
                    # kv^T per head, then Wkv_h = kv_h^T.T @ WoutT_h
                    kvT_sb = cw.tile([64, HPG, D], F32R)
                    for h in range(HPG):
                        tp = psC.tile([64, D], F32, tag="tp")
                        nc.tensor.transpose(tp, kv_sb[:, h * D:(h + 1) * D], ident[0:64, 0:64])
                        nc.vector.tensor_copy(kvT_sb[:, h, :], tp)
                    for h in range(HPG):
                        for half in range(2):
                            wps = psC.tile([64, 512], F32, tag="wps")
                            nc.tensor.matmul(
                                wps, kvT_sb[:, h, :],
                                woT_sb[:, h, half * 512:(half + 1) * 512],
                                start=True, stop=True)
                            nc.vector.tensor_copy(
                                wkv_sb[(h % 2) * 64:(h % 2) * 64 + 64, h // 2,
                                       half * 512:(half + 1) * 512], wps)
                    # ksum flatten + row-broadcast
                    ksT = psC.tile([HPG, 64], F32, tag="ksT")
                    nc.tensor.transpose(ksT, ks_sb, ident[0:64, 0:64])
                    ksT_sb = cw.tile([HPG, 64], F32)
                    nc.vector.tensor_copy(ksT_sb, ksT)
                    ks_row = cw.tile([1, DH], F32)
                    nc.sync.dma_start(ks_row, ksT_sb)               # cross-partition flatten
                    ksb_ps = psC.tile([128, DH], F32, tag="ksb")
                    nc.tensor.matmul(ksb_ps, ones_row, ks_row, start=True, stop=True)
                    nc.vector.tensor_copy(ksumb, ksb_ps)
                    if debug:
                        nc.sync.dma_start(dbg_kv.ap(), kv_sb)
                        nc.sync.dma_start(dbg_ksumb.ap(), ksumb)
                        nc.sync.dma_start(dbg_wkv.ap().bitcast(F32R),
                                          wkv_sb.rearrange("p b e -> p (b e)"))
                    kss = cw.tile([128, HPG], F32)
                    nc.vector.tensor_reduce(kss, _seg(ksumb, HPG), AXX, ALU.add)
                    nc.vector.tensor_scalar_mul(epskss, kss, EPS)

        # ---------------- phase Q: proj_q, qf, denom, y ----------------------
        with tc.tile_pool(name="wq", bufs=1) as wqp, \
             tc.tile_pool(name="qslab", bufs=2) as qslab, \
             tc.tile_pool(name="qtmp", bufs=3) as qtmp, \
             tc.tile_pool(name="qsm", bufs=4) as qsm, \
             tc.tile_pool(name="yout", bufs=3) as yout, \
             tc.tile_pool(name="psQ", bufs=2, space="PSUM") as psQ, \
             tc.tile_pool(name="psT", bufs=2, space="PSUM") as psT, \
             tc.tile_pool(name="psY", bufs=2, space="PSUM") as psY:
            w2q_sb = wqp.tile([128, JT, DH], F32R)
            nc.sync.dma_start(w2q_sb, w2q_d.ap().rearrange("(jt p) n -> p jt n", p=128))

            for ls in range(NSLAB):
                slab = qslab.tile([128, JT, SLAB], F32R, tag="slab")
                nc.sync.dma_start(slab, qT[:, :, ls * SLAB:(ls + 1) * SLAB])
                for t in range(SLAB // LT):
                    ti = ls * (SLAB // LT) + t
                    pq = psQ.tile([128, DH], F32, tag="pq")
                    for jt in range(JT):
                        nc.tensor.matmul(pq, slab[:, jt, t * LT:(t + 1) * LT],
                                         w2q_sb[:, jt, :],
                                         start=(jt == 0), stop=(not use_bias and jt == JT - 1))
                    if use_bias:
                        nc.tensor.matmul(pq, ones_row_r, bias_sb[:, 1, :], start=False, stop=True)
                    nd = qsm.tile([128, HPG], F32, tag="nd")
                    nc.vector.tensor_reduce(nd, _seg(pq, HPG), AXX, ALU.max, negate=True)
                    et = qtmp.tile([128, DH], F32, tag="et")
                    nc.vector.tensor_tensor(out=_seg(et, HPG), in0=_seg(pq, HPG),
                                            in1=_bc(nd, True, D), op=ALU.add)
                    e_sb = qtmp.tile([128, DH], F32, tag="es")
                    nc.scalar.activation(e_sb, et, EXP)             # ACT
                    prod = qtmp.tile([128, DH], F32, tag="pr")
                    nc.vector.tensor_tensor(out=prod, in0=e_sb, in1=ksumb, op=ALU.mult)
                    pre = qsm.tile([128, HPG], F32, tag="pre")
                    nc.vector.tensor_reduce(pre, _seg(prod, HPG), AXX, ALU.add)
                    den = qsm.tile([128, HPG], F32, tag="den")
                    nc.vector.scalar_tensor_tensor(out=den, in0=pre, scalar=CM,
                                                   in1=epskss, op0=ALU.mult, op1=ALU.add)
                    dnv = qsm.tile([128, HPG], F32, tag="dnv")
                    nc.vector.tensor_scalar_max(dnv, den, STAB)
                    nc.vector.reciprocal(dnv, dnv)
                    s1 = qsm.tile([128, HPG], F32, tag="s1")
                    nc.vector.tensor_scalar_mul(s1, dnv, CM)
                    qfs = qtmp.tile([128, DH], F32, tag="qfs")
                    nc.vector.scalar_tensor_tensor(
                        out=_seg(qfs, HPG), in0=_seg(e_sb, HPG), scalar=float(EPS / CM),
                        in1=_bc(s1, True, D), op0=ALU.add, op1=ALU.mult)
                    qt_sb = qtmp.tile([128, 4, 128], F32R, tag="qt")
                    for b in range(4):
                        tps = psT.tile([128, 128], F32, tag="tps")
                        nc.tensor.transpose(tps, qfs[:, b * 128:(b + 1) * 128], ident)
                        nc.vector.tensor_copy(qt_sb[:, b, :], tps)
                    if debug and ti == 0:
                        nc.sync.dma_start(dbg_qfs0.ap(), qfs)
                        nc.sync.dma_start(dbg_qt0.ap().bitcast(F32R), qt_sb.rearrange("p b l -> p (b l)"))
                    py = psY.tile([128, E], F32, tag="py")
                    for b in range(4):
                        for half in range(2):
                            nc.tensor.matmul(py[:, half * 512:(half + 1) * 512],
                                             qt_sb[:, b, :],
                                             wkv_sb[:, b, half * 512:(half + 1) * 512],
                                             start=(b == 0), stop=(b == 3))
                    y_sb = yout.tile([128, E], F32, tag="y")
                    nc.scalar.copy(y_sb[:, 0:512], py[:, 0:512])    # ACT
                    nc.vector.tensor_copy(y_sb[:, 512:], py[:, 512:])
                    nc.sync.dma_start(y_d.ap()[ti * LT:(ti + 1) * LT, :], y_sb)

    nc.compile()
    return nc


_PROGRAMS = {}


def _get_program(use_bias: bool, debug: bool = False):
    key = (use_bias, debug)
    if key not in _PROGRAMS:
        _PROGRAMS[key] = _build_program(use_bias, debug)
    return _PROGRAMS[key]


def _make_orthogonal(mat):
    q, r = np.linalg.qr(np.swapaxes(mat, -2, -1))
    d = np.diagonal(r, 0, -2, -1)[..., None]
    q = q * np.sign(d)
    return np.swapaxes(q, -2, -1).astype(np.float32)


def _prep(query, key, value, in_proj_weight, in_proj_bias, out_proj_weight,
          projection_matrix):
    c4 = np.float32(D ** -0.25)
    P = _make_orthogonal(np.asarray(projection_matrix, np.float32))
    ipw = np.asarray(in_proj_weight, np.float32)
    wq, wk, wv = ipw[:E], ipw[E:2 * E], ipw[2 * E:]
    Wxq = np.ascontiguousarray((c4 * wq).T)
    Wxk = np.ascontiguousarray((c4 * wk).T)
    Wv = np.ascontiguousarray(wv.T)
    W2q = np.empty((E, E), np.float32)
    W2k = np.empty((E, E), np.float32)
    for h in range(H):
        s = slice(h * D, (h + 1) * D)
        W2q[:, s] = Wxq[:, s] @ P[h]
        W2k[:, s] = Wxk[:, s] @ P[h]
    OPT = np.ascontiguousarray(np.asarray(out_proj_weight, np.float32).T)

    # transposed activations, one big pass each: [L, N, E] -> [N, E, L]
    QT = np.ascontiguousarray(np.asarray(query, np.float32).transpose(1, 2, 0))
    KT = np.ascontiguousarray(np.asarray(key, np.float32).transpose(1, 2, 0))
    VT = np.ascontiguousarray(np.asarray(value, np.float32).transpose(1, 2, 0))

    ipb = np.asarray(in_proj_bias, np.float32)
    use_bias = bool(np.any(ipb))
    bq, bk, bv = ipb[:E], ipb[E:2 * E], ipb[2 * E:]

    in_maps = []
    for core in range(8):
        n, hg = core // 2, core % 2
        cs = slice(hg * DH, (hg + 1) * DH)
        woT = np.ascontiguousarray(
            OPT[hg * DH:(hg + 1) * DH, :].reshape(HPG, D, E).transpose(1, 0, 2)
        ).reshape(D, HPG * E)
        m = {
            "qT": QT[n], "kT": KT[n], "vT": VT[n],
            "w2q": np.ascontiguousarray(W2q[:, cs]),
            "wxk": np.ascontiguousarray(Wxk[:, cs]),
            "w2k": np.ascontiguousarray(W2k[:, cs]),
            "wv": np.ascontiguousarray(Wv[:, cs]),
            "woT": woT,
        }
        if use_bias:
            bx = c4 * bk[cs]
            bp = np.concatenate([(c4 * bk[h * D:(h + 1) * D]) @ P[h]
                                 for h in range(hg * HPG, (hg + 1) * HPG)])
            m["biases"] = np.concatenate([bx, bp, bv[cs]])[None, :].astype(np.float32)
        in_maps.append(m)
    return in_maps, use_bias


def _run(inputs, trace=False, trace_kwargs=None):
    in_maps, use_bias = _prep(
        inputs["query"], inputs["key"], inputs["value"], inputs["in_proj_weight"],
        inputs["in_proj_bias"], inputs["out_proj_weight"], inputs["projection_matrix"])
    nc = _get_program(use_bias)
    kw = {}
    if trace:
        kw = dict(trace=True, **(trace_kwargs or {}))
    res = run_bass_kernel_spmd(nc, in_maps, core_ids=list(range(8)), **kw)
    opb = np.asarray(inputs["out_proj_bias"], np.float32)
    y = np.empty((L, N, E), np.float32)
    for n in range(N):
        y[:, n, :] = res.results[2 * n]["y"] + res.results[2 * n + 1]["y"] + opb
    return y, res


def kernel(**inputs) -> np.ndarray:
    y, _ = _run(inputs)
    return y


# revision 13
# speedup vs baseline: 1.1224x; 1.1224x over previous
"""Performer (FAVOR+) attention block for Trainium2, 8-core SPMD.

Sharding: core i handles batch n = i//2 and head-group hg = i%2 (8 of 16
heads). All cross-head/cross-batch reductions are local to a core; the only
cross-core step is summing the two per-head-group partial outputs of each
batch (done on host after gather).

Math restructuring (validated vs reference to ~2e-6 rel):
  - P = QR-orthogonalization of projection_matrix (host, numpy - tiny).
  - x_k = key @ Wxk  with Wxk = (D^-1/4 * wk)^T                (per head grp)
  - proj = x @ P  composed into the input projection:  W2 = Wxk @ P_h
  - query path: h = ||x||^2/2 cancels inside exp(delta - max_e delta), so
    only proj_q is needed:  qf = D^-1/2 * exp(proj - max_e proj) + eps
  - key path: kp = exp(proj - (h + diff + ln(D)/2)) + eps, diff = global max
  - out_proj composed through kv:  y = sum_h qfs_h @ (kv_h @ WoutT_h),
    qfs = qf * D_inv  (denominator applied to qf before the kv matmul)
"""
import numpy as np
from contextlib import ExitStack

import concourse.bass as bass
import concourse.tile as tile
from concourse import bacc, mybir
from concourse.bass_utils import run_bass_kernel_spmd
from concourse.masks import make_identity
from concourse import bass_isa

L, N, E, H, D = 4096, 4, 1024, 16, 64
HG = 2                 # head-groups (cores per batch)
HPG = H // HG          # 8 heads per group
DH = HPG * D           # 512 feature columns per group
LT = 128               # rows per l-tile
NLT = L // LT          # 32
SLAB = 512             # l columns per DMA slab
NSLAB = L // SLAB      # 8
JT = E // 128          # 8 contraction tiles

F32 = mybir.dt.float32
F32R = mybir.dt.float32r
EXP = mybir.ActivationFunctionType.Exp
ALU = mybir.AluOpType
AXX = mybir.AxisListType.X
AXC = mybir.AxisListType.C

CM = float(D ** -0.5)
EPS = 1e-6
STAB = 1e-6
B2C = float(0.5 * np.log(D))


def _bc(ap2, g_axis_first, reps):
    """[P, G] -> broadcast AP. g_axis_first=True: [P, G, reps] (repeat inner);
    False: [P, reps, G] (repeat middle)."""
    p_dim, g_dim = ap2.ap[0], ap2.ap[1]
    if g_axis_first:
        return bass.AP(tensor=ap2.tensor, offset=ap2.offset,
                       ap=[p_dim, g_dim, [0, reps]])
    return bass.AP(tensor=ap2.tensor, offset=ap2.offset,
                   ap=[p_dim, [0, reps], g_dim])


def _seg(ap2, g):
    """[P, G*D] tile view -> [P, G, D]."""
    return ap2.rearrange("p (g d) -> p g d", g=g)


def _build_program(use_bias: bool, debug: bool = False):
    nc = bacc.Bacc("TRN2", target_bir_lowering=False, debug=False)

    qT_d = nc.dram_tensor("qT", [E, L], F32R, kind="ExternalInput")
    kT_d = nc.dram_tensor("kT", [E, L], F32R, kind="ExternalInput")
    vT_d = nc.dram_tensor("vT", [E, L], F32R, kind="ExternalInput")
    w2q_d = nc.dram_tensor("w2q", [E, DH], F32R, kind="ExternalInput")
    w2k_d = nc.dram_tensor("w2k", [E, DH], F32R, kind="ExternalInput")
    wv_d = nc.dram_tensor("wv", [E, DH], F32R, kind="ExternalInput")
    woT_d = nc.dram_tensor("woT", [D, HPG * E], F32R, kind="ExternalInput")
    if use_bias:
        bias_d = nc.dram_tensor("biases", [1, 3 * DH], F32R, kind="ExternalInput")
    y_d = nc.dram_tensor("y", [L, E], F32, kind="ExternalOutput")
    if debug:
        dbg_hd = nc.dram_tensor("dbg_hd", [128, NLT * HPG], F32, kind="ExternalOutput")
        dbg_delta0 = nc.dram_tensor("dbg_delta0", [128, DH], F32, kind="ExternalOutput")
        dbg_kp0 = nc.dram_tensor("dbg_kp0", [128, DH], F32, kind="ExternalOutput")
        dbg_v0 = nc.dram_tensor("dbg_v0", [128, DH], F32, kind="ExternalOutput")
        dbg_kv = nc.dram_tensor("dbg_kv", [64, DH], F32, kind="ExternalOutput")
        dbg_ksumb = nc.dram_tensor("dbg_ksumb", [128, DH], F32, kind="ExternalOutput")
        dbg_wkv = nc.dram_tensor("dbg_wkv", [128, 4 * E], F32, kind="ExternalOutput")
        dbg_qfs0 = nc.dram_tensor("dbg_qfs0", [128, DH], F32, kind="ExternalOutput")
        dbg_qt0 = nc.dram_tensor("dbg_qt0", [128, 4 * 128], F32, kind="ExternalOutput")

    qT = qT_d.ap().rearrange("(jt p) l -> p jt l", p=128)
    kT = kT_d.ap().rearrange("(jt p) l -> p jt l", p=128)
    vT = vT_d.ap().rearrange("(jt p) l -> p jt l", p=128)

    with tile.TileContext(nc) as tc, ExitStack() as ctx:
        const = ctx.enter_context(tc.tile_pool(name="const", bufs=1))
        pers = ctx.enter_context(tc.tile_pool(name="pers", bufs=1))

        ident = const.tile([128, 128], F32)
        make_identity(nc, ident)
        ones_col = const.tile([128, 1], F32)
        nc.vector.memset(ones_col, 1.0)
        ones_row = const.tile([1, 128], F32)
        nc.vector.memset(ones_row, 1.0)
        if use_bias:
            bias_sb = const.tile([1, 3, DH], F32R)
            nc.sync.dma_start(bias_sb, bias_d.ap().rearrange("o (t n) -> o t n", t=3))
            ones_row_r = const.tile([1, 128], F32R)
            nc.vector.memset(ones_row_r, 1.0)

        # persistent across phases
        hd_all = pers.tile([128, NLT, HPG], F32)      # h-sums, then 0.5h+b2
        runmax = pers.tile([128, HPG], F32)
        ksumb = pers.tile([128, DH], F32)             # ksum broadcast to rows
        epskss = pers.tile([128, HPG], F32)
        wkv_sb = pers.tile([128, 4, E], F32R)         # stacked head-pair kv@WoutT

        nc.vector.memset(runmax, -1e30)

        # ---------------- phase K1: x_k, proj_k, h, running max --------------
        with tc.tile_pool(name="deltap", bufs=1) as deltap:
            delta_all = deltap.tile([128, NLT, DH], F32)   # stores proj_k

            with tc.tile_pool(name="wk", bufs=1) as wkp, \
                 tc.tile_pool(name="kslab", bufs=2) as kslab, \
                 tc.tile_pool(name="k1tmp", bufs=3) as k1tmp, \
                 tc.tile_pool(name="k1sm", bufs=4) as k1sm, \
                 tc.tile_pool(name="psK1", bufs=2, space="PSUM") as psK1:
                w2k_sb = wkp.tile([128, JT, DH], F32R)
                nc.sync.dma_start(w2k_sb, w2k_d.ap().rearrange("(jt p) n -> p jt n", p=128))

                for ls in range(NSLAB):
                    slab = kslab.tile([128, JT, SLAB], F32R, tag="slab")
                    nc.sync.dma_start(slab, kT[:, :, ls * SLAB:(ls + 1) * SLAB])
                    for t in range(SLAB // LT):
                        ti = ls * (SLAB // LT) + t
                        pp = psK1.tile([128, DH], F32, tag="pp")
                        for jt in range(JT):
                            lhsT = slab[:, jt, t * LT:(t + 1) * LT]
                            nc.tensor.matmul(pp, lhsT, w2k_sb[:, jt, :],
                                             start=(jt == 0), stop=(not use_bias and jt == JT - 1))
                        if use_bias:
                            nc.tensor.matmul(pp, ones_row_r, bias_sb[:, 1, :], start=False, stop=True)
                        # P is orthogonal, so ||x||^2 == ||proj||^2: h from proj
                        sq = k1tmp.tile([128, DH], F32, tag="sq")
                        nc.scalar.activation(sq, pp, mybir.ActivationFunctionType.Square)
                        nc.vector.tensor_reduce(hd_all[:, ti, :], _seg(sq, HPG), AXX, ALU.add)
                        nc.scalar.copy(delta_all[:, ti, :], pp)     # ACT: psum->sbuf
                        rm = k1sm.tile([128, HPG], F32, tag="rm")
                        nc.vector.tensor_reduce(rm, _seg(pp, HPG), AXX, ALU.max)
                        td = k1sm.tile([128, HPG], F32, tag="td")
                        nc.vector.scalar_tensor_tensor(out=td, in0=hd_all[:, ti, :],
                                                       scalar=-0.5, in1=rm,
                                                       op0=ALU.mult, op1=ALU.add)
                        nc.vector.tensor_tensor(out=runmax, in0=runmax, in1=td, op=ALU.max)

            # ---------------- phase K1.5: diff, b2, hd2 ----------------------
            with tc.tile_pool(name="k15", bufs=1) as k15:
                diffb = k15.tile([128, HPG], F32)
                nc.gpsimd.partition_all_reduce(diffb, runmax, 128,
                                               bass_isa.ReduceOp.max)
                b2cols = k15.tile([128, HPG], F32)
                nc.vector.tensor_scalar_add(b2cols, diffb, B2C)
                # hd2 = 0.5*hsum + b2   (one op over the whole store)
                nc.vector.scalar_tensor_tensor(
                    out=hd_all.rearrange("p t g -> p (t g)"),
                    in0=hd_all.rearrange("p t g -> p (t g)"),
                    scalar=0.5,
                    in1=_bc(b2cols, False, NLT),
                    op0=ALU.mult, op1=ALU.add)

            # ---------------- phase K2: v, kp, kv/ksum accumulation ----------
            with tc.tile_pool(name="psAcc", bufs=1, space="PSUM") as psAcc:
                kvps = psAcc.tile([64, DH], F32, tag="kv")
                ksps = psAcc.tile([64, HPG], F32, tag="ks")
                with tc.tile_pool(name="wv", bufs=1) as wvp, \
                     tc.tile_pool(name="vslab", bufs=2) as vslab, \
                     tc.tile_pool(name="k2tmp", bufs=3) as k2tmp, \
                     tc.tile_pool(name="psK2", bufs=2, space="PSUM") as psK2:
                    wv_sb = wvp.tile([128, JT, DH], F32R)
                    nc.sync.dma_start(wv_sb, wv_d.ap().rearrange("(jt p) n -> p jt n", p=128))
                    for ls in range(NSLAB):
                        slab = vslab.tile([128, JT, SLAB], F32R, tag="slab")
                        nc.sync.dma_start(slab, vT[:, :, ls * SLAB:(ls + 1) * SLAB])
                        for t in range(SLAB // LT):
                            ti = ls * (SLAB // LT) + t
                            pv = psK2.tile([128, DH], F32, tag="pv")
                            for jt in range(JT):
                                nc.tensor.matmul(pv, slab[:, jt, t * LT:(t + 1) * LT],
                                                 wv_sb[:, jt, :],
                                                 start=(jt == 0), stop=(not use_bias and jt == JT - 1))
                            if use_bias:
                                nc.tensor.matmul(pv, ones_row_r, bias_sb[:, 2, :], start=False, stop=True)
                            v_sb = k2tmp.tile([128, DH], F32, tag="v")
                            nc.scalar.copy(v_sb, pv)                    # ACT
                            kpe = k2tmp.tile([128, DH], F32, tag="kpe")
                            nc.vector.tensor_tensor(
                                out=_seg(kpe, HPG), in0=_seg(delta_all[:, ti, :], HPG),
                                in1=_bc(hd_all[:, ti, :], True, D), op=ALU.subtract)
                            kx = k2tmp.tile([128, DH], F32, tag="kx")
                            nc.scalar.activation(kx, kpe, EXP)          # ACT
                            kp_sb = k2tmp.tile([128, DH], F32, tag="kp")
                            nc.vector.tensor_scalar_add(kp_sb, kx, EPS)
                            if debug and ti == 0:
                                nc.sync.dma_start(dbg_kp0.ap(), kp_sb)
                                nc.sync.dma_start(dbg_v0.ap(), v_sb)
                                nc.sync.dma_start(dbg_delta0.ap(), delta_all[:, 0, :])
                                nc.sync.dma_start(dbg_hd.ap(), hd_all.rearrange("p t g -> p (t g)"))
                            last = (ti == NLT - 1)
                            for h in range(HPG):
                                hs = slice(h * D, (h + 1) * D)
                                # start=True clears the whole PSUM bank, so only
                                # the first matmul touching each accumulator may
                                # set it; later regions overwrite-on-clear.
                                nc.tensor.matmul(kvps[:, hs], kp_sb[:, hs], v_sb[:, hs],
                                                 start=(ti == 0 and h == 0),
                                                 stop=(last and h == HPG - 1))
                                nc.tensor.matmul(ksps[:, h:h + 1], kp_sb[:, hs], ones_col,
                                                 start=(ti == 0 and h == 0),
                                                 stop=(last and h == HPG - 1))

                # ---------------- phase C: compose Wkv, ksum broadcast -------
                with tc.tile_pool(name="cw", bufs=1) as cw, \
                     tc.tile_pool(name="psC", bufs=1, space="PSUM") as psC:
                    woT_sb = cw.tile([64, HPG, E], F32R)
                    nc.sync.dma_start(woT_sb, woT_d.ap().rearrange("d (g e) -> d g e", g=HPG))
                    kv_sb = cw.tile([64, DH], F32)
                    nc.vector.tensor_copy(kv_sb, kvps)
                    ks_sb = cw.tile([64, HPG], F32)
                    nc.vector.tensor_copy(ks_sb, ksps)
                    # kv^T per head, then Wkv_h = kv_h^T.T @ WoutT_h
                    kvT_sb = cw.tile([64, HPG, D], F32R)
                    for h in range(HPG):
                        tp = psC.tile([64, D], F32, tag="tp")
                        nc.tensor.transpose(tp, kv_sb[:, h * D:(h + 1) * D], ident[0:64, 0:64])
                        nc.vector.tensor_copy(kvT_sb[:, h, :], tp)
                    for h in range(HPG):
                        for half in range(2):
                            wps = psC.tile([64, 512], F32, tag="wps")
                            nc.tensor.matmul(
                                wps, kvT_sb[:, h, :],
                                woT_sb[:, h, half * 512:(half + 1) * 512],
                                start=True, stop=True)
                            nc.vector.tensor_copy(
                                wkv_sb[(h % 2) * 64:(h % 2) * 64 + 64, h // 2,
                                       half * 512:(half + 1) * 512], wps)
                    # ksum flatten + row-broadcast
                    ksT = psC.tile([HPG, 64], F32, tag="ksT")
                    nc.tensor.transpose(ksT, ks_sb, ident[0:64, 0:64])
                    ksT_sb = cw.tile([HPG, 64], F32)
                    nc.vector.tensor_copy(ksT_sb, ksT)
                    ks_row = cw.tile([1, DH], F32)
                    nc.sync.dma_start(ks_row, ksT_sb)               # cross-partition flatten
                    ksb_ps = psC.tile([128, DH], F32, tag="ksb")
                    nc.tensor.matmul(ksb_ps, ones_row, ks_row, start=True, stop=True)
                    nc.vector.tensor_copy(ksumb, ksb_ps)
                    if debug:
                        nc.sync.dma_start(dbg_kv.ap(), kv_sb)
                        nc.sync.dma_start(dbg_ksumb.ap(), ksumb)
                        nc.sync.dma_start(dbg_wkv.ap().bitcast(F32R),
                                          wkv_sb.rearrange("p b e -> p (b e)"))
                    kss = cw.tile([128, HPG], F32)
                    nc.vector.tensor_reduce(kss, _seg(ksumb, HPG), AXX, ALU.add)
                    nc.vector.tensor_scalar_mul(epskss, kss, EPS)

        # ---------------- phase Q: proj_q, qf, denom, y ----------------------
        with tc.tile_pool(name="wq", bufs=1) as wqp, \
             tc.tile_pool(name="qslab", bufs=2) as qslab, \
             tc.tile_pool(name="qtmp", bufs=3) as qtmp, \
             tc.tile_pool(name="qsm", bufs=4) as qsm, \
             tc.tile_pool(name="yout", bufs=3) as yout, \
             tc.tile_pool(name="psQ", bufs=2, space="PSUM") as psQ, \
             tc.tile_pool(name="psT", bufs=2, space="PSUM") as psT, \
             tc.tile_pool(name="psY", bufs=2, space="PSUM") as psY:
            w2q_sb = wqp.tile([128, JT, DH], F32R)
            nc.sync.dma_start(w2q_sb, w2q_d.ap().rearrange("(jt p) n -> p jt n", p=128))

            for ls in range(NSLAB):
                slab = qslab.tile([128, JT, SLAB], F32R, tag="slab")
                nc.sync.dma_start(slab, qT[:, :, ls * SLAB:(ls + 1) * SLAB])
                for t in range(SLAB // LT):
                    ti = ls * (SLAB // LT) + t
                    pq = psQ.tile([128, DH], F32, tag="pq")
                    for jt in range(JT):
                        nc.tensor.matmul(pq, slab[:, jt, t * LT:(t + 1) * LT],
                                         w2q_sb[:, jt, :],
                                         start=(jt == 0), stop=(not use_bias and jt == JT - 1))
                    if use_bias:
                        nc.tensor.matmul(pq, ones_row_r, bias_sb[:, 1, :], start=False, stop=True)
                    nd = qsm.tile([128, HPG], F32, tag="nd")
                    nc.vector.tensor_reduce(nd, _seg(pq, HPG), AXX, ALU.max, negate=True)
                    et = qtmp.tile([128, DH], F32, tag="et")
                    nc.vector.tensor_tensor(out=_seg(et, HPG), in0=_seg(pq, HPG),
                                            in1=_bc(nd, True, D), op=ALU.add)
                    e_sb = qtmp.tile([128, DH], F32, tag="es")
                    nc.scalar.activation(e_sb, et, EXP)             # ACT
                    prod = qtmp.tile([128, DH], F32, tag="pr")
                    nc.vector.tensor_tensor(out=prod, in0=e_sb, in1=ksumb, op=ALU.mult)
                    pre = qsm.tile([128, HPG], F32, tag="pre")
                    nc.vector.tensor_reduce(pre, _seg(prod, HPG), AXX, ALU.add)
                    den = qsm.tile([128, HPG], F32, tag="den")
                    nc.vector.scalar_tensor_tensor(out=den, in0=pre, scalar=CM,
                                                   in1=epskss, op0=ALU.mult, op1=ALU.add)
                    dnv = qsm.tile([128, HPG], F32, tag="dnv")
                    nc.vector.tensor_scalar_max(dnv, den, STAB)
                    nc.vector.reciprocal(dnv, dnv)
                    s1 = qsm.tile([128, HPG], F32, tag="s1")
                    nc.vector.tensor_scalar_mul(s1, dnv, CM)
                    qfs = qtmp.tile([128, DH], F32, tag="qfs")
                    nc.vector.scalar_tensor_tensor(
                        out=_seg(qfs, HPG), in0=_seg(e_sb, HPG), scalar=float(EPS / CM),
                        in1=_bc(s1, True, D), op0=ALU.add, op1=ALU.mult)
                    qt_sb = qtmp.tile([128, 4, 128], F32R, tag="qt")
                    for b in range(4):
                        tps = psT.tile([128, 128], F32, tag="tps")
                        nc.tensor.transpose(tps, qfs[:, b * 128:(b + 1) * 128], ident)
                        nc.vector.tensor_copy(qt_sb[:, b, :], tps)
                    if debug and ti == 0:
                        nc.sync.dma_start(dbg_qfs0.ap(), qfs)
                        nc.sync.dma_start(dbg_qt0.ap().bitcast(F32R), qt_sb.rearrange("p b l -> p (b l)"))
                    py = psY.tile([128, E], F32, tag="py")
                    for b in range(4):
                        for half in range(2):
                            nc.tensor.matmul(py[:, half * 512:(half + 1) * 512],
                                             qt_sb[:, b, :],
                                             wkv_sb[:, b, half * 512:(half + 1) * 512],
                                             start=(b == 0), stop=(b == 3))
                    y_sb = yout.tile([128, E], F32, tag="y")
                    nc.scalar.copy(y_sb[:, 0:512], py[:, 0:512])    # ACT
                    nc.vector.tensor_copy(y_sb[:, 512:], py[:, 512:])
                    nc.sync.dma_start(y_d.ap()[ti * LT:(ti + 1) * LT, :], y_sb)

    nc.compile()
    return nc


_PROGRAMS = {}


def _get_program(use_bias: bool, debug: bool = False):
    key = (use_bias, debug)
    if key not in _PROGRAMS:
        _PROGRAMS[key] = _build_program(use_bias, debug)
    return _PROGRAMS[key]


def _make_orthogonal(mat):
    q, r = np.linalg.qr(np.swapaxes(mat, -2, -1))
    d = np.diagonal(r, 0, -2, -1)[..., None]
    q = q * np.sign(d)
    return np.swapaxes(q, -2, -1).astype(np.float32)


def _prep(query, key, value, in_proj_weight, in_proj_bias, out_proj_weight,
          projection_matrix):
    c4 = np.float32(D ** -0.25)
    P = _make_orthogonal(np.asarray(projection_matrix, np.float32))
    ipw = np.asarray(in_proj_weight, np.float32)
    wq, wk, wv = ipw[:E], ipw[E:2 * E], ipw[2 * E:]
    Wxq = np.ascontiguousarray((c4 * wq).T)
    Wxk = np.ascontiguousarray((c4 * wk).T)
    Wv = np.ascontiguousarray(wv.T)
    W2q = np.empty((E, E), np.float32)
    W2k = np.empty((E, E), np.float32)
    for h in range(H):
        s = slice(h * D, (h + 1) * D)
        W2q[:, s] = Wxq[:, s] @ P[h]
        W2k[:, s] = Wxk[:, s] @ P[h]
    OPT = np.ascontiguousarray(np.asarray(out_proj_weight, np.float32).T)

    # transposed activations, one big pass each: [L, N, E] -> [N, E, L]
    QT = np.ascontiguousarray(np.asarray(query, np.float32).transpose(1, 2, 0))
    KT = np.ascontiguousarray(np.asarray(key, np.float32).transpose(1, 2, 0))
    VT = np.ascontiguousarray(np.asarray(value, np.float32).transpose(1, 2, 0))

    ipb = np.asarray(in_proj_bias, np.float32)
    use_bias = bool(np.any(ipb))
    bq, bk, bv = ipb[:E], ipb[E:2 * E], ipb[2 * E:]

    in_maps = []
    for core in range(8):
        n, hg = core // 2, core % 2
        cs = slice(hg * DH, (hg + 1) * DH)
        woT = np.ascontiguousarray(
            OPT[hg * DH:(hg + 1) * DH, :].reshape(HPG, D, E).transpose(1, 0, 2)
        ).reshape(D, HPG * E)
        m = {
            "qT": QT[n], "kT": KT[n], "vT": VT[n],
            "w2q": np.ascontiguousarray(W2q[:, cs]),
            "w2k": np.ascontiguousarray(W2k[:, cs]),
            "wv": np.ascontiguousarray(Wv[:, cs]),
            "woT": woT,
        }
        if use_bias:
            bx = c4 * bk[cs]
            bp = np.concatenate([(c4 * bk[h * D:(h + 1) * D]) @ P[h]
                                 for h in range(hg * HPG, (hg + 1) * HPG)])
            m["biases"] = np.concatenate([bx, bp, bv[cs]])[None, :].astype(np.float32)
        in_maps.append(m)
    return in_maps, use_bias


def _run(inputs, trace=False, trace_kwargs=None):
    in_maps, use_bias = _prep(
        inputs["query"], inputs["key"], inputs["value"], inputs["in_proj_weight"],
        inputs["in_proj_bias"], inputs["out_proj_weight"], inputs["projection_matrix"])
    nc = _get_program(use_bias)
    kw = {}
    if trace:
        kw = dict(trace=True, **(trace_kwargs or {}))
    res = run_bass_kernel_spmd(nc, in_maps, core_ids=list(range(8)), **kw)
    opb = np.asarray(inputs["out_proj_bias"], np.float32)
    y = np.empty((L, N, E), np.float32)
    for n in range(N):
        y[:, n, :] = res.results[2 * n]["y"] + res.results[2 * n + 1]["y"] + opb
    return y, res


def kernel(**inputs) -> np.ndarray:
    y, _ = _run(inputs)
    return y
